# revision 9
# baseline (speedup 1.0000x reference)
"""GraphSAGE edge layer (sigmoid-gated message passing + segment-max) on 8 Trainium2
NeuronCores via Bass/Tile.

Strategy (graph/data parallel):
  - Nodes are sharded across 8 cores (6250 each); edges partitioned by destination
    node so the segment-max reduce is core-local.
  - Per core, a replicated G table [N, 256] = [Ah | Bh] (bf16) is built on-device
    (node-major rows so the per-edge indirect gather reads one 512B row per slot).
    PSUM->SBUF copies alternate between Vector and Scalar so neither engine gates
    the PE.
  - Edges are packed host-side into a degree-bucketed (node x slot) grid with
    uniform slots-per-node within each gather group (1-4 blocks of 128 nodes).
    Pad slots point at a zero row of G, whose Ah half is 0 => gated contribution
    is exactly 0, absorbed by the final clamp.
  - Bh[dst] for local nodes is computed directly into SBUF (no gather needed).
  - Per group: ONE indirect DMA gathers all slot rows; the gate is computed
    in-place in the gathered tile (add on Vector, sigmoid on Scalar, mult on
    GpSimd); the segment max runs as a contiguous in-place max tree on Vector
    with the 0-clamp fused into the last level via scalar_tensor_tensor.
  - Update MLP runs per block in bf16 (c^T via one PE transpose); sqrt/normalize
    is deferred to one batched tail pass so the Scalar engine never swaps
    activation tables during the main loop.
"""

import numpy as np
import ml_dtypes

from concourse import bass, bacc, mybir
from concourse.tile import TileContext
from concourse.bass_utils import run_bass_kernel_spmd
from concourse.masks import make_identity

BF16 = ml_dtypes.bfloat16

N = 50000
E = 800000
D = 128
NCORES = 8
NLOC = N // NCORES          # 6250 nodes per core
P = 128
NBLK1 = (N + P - 1) // P    # 391 G-build blocks
NPADG = NBLK1 * P           # 50048 (pad/zero row of G lives at this index)
NBLK = (NLOC + P - 1) // P  # 49 local node blocks
NLOCP = NBLK * P            # 6272
EPS = 1e-12
CH1 = 16                    # G-build hT chunk (blocks per DMA)
GW1 = 8                     # G-build write chunk (blocks per DMA)
GCAP = 48                   # max slots per gather group (48*512B = 24KB/partition)
OBW = 8                     # output write batch (blocks)

_prog_cache = {}


# --------------------------------------------------------------------------- host


def _preprocess(h, src, dst):
    """Shard edges by destination, build per-core degree-sorted slot grids.

    Blocks are grouped (1-4 per gather) with a uniform slot count kg per group.
    Returns (groups, total_cols, per_core) where groups[i] = (b0, n, kg, off)
    and per_core[c] = (gidx [128, total_cols] int32, perm [NLOC] int64).
    """
    src = np.asarray(src).astype(np.int64)
    dst = np.asarray(dst).astype(np.int64)

    order = np.argsort(dst, kind="stable")
    dst_s = dst[order]
    src_s = src[order]
    bounds = np.searchsorted(dst_s, np.arange(NCORES + 1) * NLOC)

    cores = []
    blkmax = np.zeros((NCORES, NBLK), np.int64)
    for c in range(NCORES):
        a, b = bounds[c], bounds[c + 1]
        ldst = dst_s[a:b] - c * NLOC
        lsrc = src_s[a:b]
        deg = np.bincount(ldst, minlength=NLOC)
        perm = np.argsort(-deg, kind="stable")
        pdeg = np.zeros(NLOCP, np.int64)
        pdeg[:NLOC] = deg[perm]
        blkmax[c] = pdeg.reshape(NBLK, P).max(axis=1)
        cores.append((lsrc, deg, perm))

    K = blkmax.max(axis=0)
    K = np.maximum((K + 1) // 2 * 2, 2).astype(np.int64)  # even, >= 2

    groups = []
    b = 0
    off = 0
    while b < NBLK:
        kg = int(K[b])
        n = max(1, min(4, NBLK - b, GCAP // kg))
        groups.append((b, n, kg, off))
        off += n * kg
        b += n
    total_cols = int(off)

    col0 = np.zeros(NBLK, np.int64)
    for (b0, n, kg, goff) in groups:
        for j in range(n):
            col0[b0 + j] = goff + j * kg

    per_core = []
    for c in range(NCORES):
        lsrc, deg, perm = cores[c]
        start = np.concatenate([[0], np.cumsum(deg)])
        gidx = np.full((P, total_cols), NPADG, np.int32)
        for blk in range(NBLK):
            o = int(col0[blk])
            for p in range(P):
                i = blk * P + p
                if i >= NLOC:
                    continue
                node = int(perm[i])
                dn = int(deg[node])
                if dn:
                    gidx[p, o : o + dn] = lsrc[start[node] : start[node] + dn]
        per_core.append((gidx, perm))
    return groups, total_cols, per_core


# --------------------------------------------------------------------------- bass


def _build(groups, total_cols):
    f32 = mybir.dt.float32
    bf16 = mybir.dt.bfloat16
    i32 = mybir.dt.int32
    Sig = mybir.ActivationFunctionType.Sigmoid
    Relu = mybir.ActivationFunctionType.Relu
    mx = mybir.AluOpType.max
    ad = mybir.AluOpType.add
    ml = mybir.AluOpType.mult

    nc = bacc.Bacc(
        "TRN2", target_bir_lowering=False, debug=False, num_devices=NCORES
    )
    hT = nc.declare_dram_parameter("hT", [D, NPADG], bf16, isOutput=False)
    hloc = nc.declare_dram_parameter("hloc", [NLOCP, D], f32, isOutput=False)
    hTloc = nc.declare_dram_parameter("hTloc", [D, NLOCP], bf16, isOutput=False)
    gidx = nc.declare_dram_parameter("gidx", [P, total_cols], i32, isOutput=False)
    wcat = nc.declare_dram_parameter("wcat", [D, 2 * D], bf16, isOutput=False)
    brhs = nc.declare_dram_parameter("brhs", [1, 2 * D], bf16, isOutput=False)
    u1 = nc.declare_dram_parameter("u1", [D, D], bf16, isOutput=False)
    u2 = nc.declare_dram_parameter("u2", [D, D], bf16, isOutput=False)
    ubr = nc.declare_dram_parameter("ubr", [1, D], bf16, isOutput=False)
    outp = nc.declare_dram_parameter("out", [NLOCP, D], f32, isOutput=True)

    G = nc.dram_tensor("G", [NPADG + 1, 2 * D], bf16)

    with TileContext(nc) as tc:
        with (
            tc.tile_pool(name="const", bufs=1) as cpool,
            tc.tile_pool(name="ht1", bufs=3) as htpool,
            tc.tile_pool(name="gs1", bufs=3) as gspool,
            tc.tile_pool(name="gat", bufs=2) as gatpool,
            tc.tile_pool(name="sc", bufs=3) as scpool,
            tc.tile_pool(name="ob", bufs=2) as obpool,
            tc.tile_pool(name="psA", bufs=4, space="PSUM") as psA,
            tc.tile_pool(name="psB", bufs=2, space="PSUM") as psB,
        ):
            # ---- constants
            wcat_t = cpool.tile([D, 2 * D], bf16)
            nc.sync.dma_start(out=wcat_t[:], in_=wcat[:, :])
            brhs_t = cpool.tile([1, 2 * D], bf16)
            nc.sync.dma_start(out=brhs_t[:], in_=brhs[:, :])
            u1_t = cpool.tile([D, D], bf16)
            nc.sync.dma_start(out=u1_t[:], in_=u1[:, :])
            u2_t = cpool.tile([D, D], bf16)
            nc.sync.dma_start(out=u2_t[:], in_=u2[:, :])
            ubr_t = cpool.tile([1, D], bf16)
            nc.sync.dma_start(out=ubr_t[:], in_=ubr[:, :])
            idx_t = cpool.tile([P, total_cols], i32)
            nc.sync.dma_start(out=idx_t[:], in_=gidx[:, :])

            ones_bf = cpool.tile([1, P], bf16)
            nc.vector.memset(ones_bf[:], 1.0)
            ident = cpool.tile([P, P], bf16)
            make_identity(nc, ident[:])

            zrow = cpool.tile([1, 2 * D], bf16)
            nc.vector.memset(zrow[:], 0.0)
            nc.sync.dma_start(out=G[NPADG : NPADG + 1, :], in_=zrow[:])

            # ---- preload per-core local h (both layouts) once
            hloc_sb = cpool.tile([P, NBLK * D], f32)
            nc.sync.dma_start(
                out=hloc_sb[:].rearrange("p (b x) -> p b x", b=NBLK),
                in_=hloc[:, :].rearrange("(b p) x -> p b x", p=P),
            )
            hTloc_sb = cpool.tile([D, NLOCP], bf16)
            nc.sync.dma_start(out=hTloc_sb[:], in_=hTloc[:, :])

            bhloc_sb = cpool.tile([P, NBLK * D], bf16)
            cball = cpool.tile([P, NBLK * D], bf16)
            bpall = cpool.tile([P, NBLK * D], f32)
            ssqall = cpool.tile([P, NBLK], f32)
            nrm_t = cpool.tile([P, NBLK], f32)
            rn_t = cpool.tile([P, NBLK], f32)

            # ---- phase 1: G = [h @ A_w + A_b | h @ B_w + B_b]  (bf16)
            ncopy = 0
            for c0 in range(0, NBLK1, CH1):
                nb = min(CH1, NBLK1 - c0)
                ht_t = htpool.tile([D, CH1 * P], bf16, tag="ht")
                nc.sync.dma_start(
                    out=ht_t[:, : nb * P],
                    in_=hT[:, c0 * P : (c0 + nb) * P],
                )
                for g0 in range(0, nb, GW1):
                    gn = min(GW1, nb - g0)
                    gs = gspool.tile([P, GW1 * 2 * D], bf16, tag="gs")
                    for j in range(g0, g0 + gn):
                        ps = psA.tile([P, 2 * D], f32, tag="g")
                        nc.tensor.matmul(
                            out=ps[:],
                            lhsT=ones_bf[:],
                            rhs=brhs_t[:],
                            start=True,
                            stop=False,
                        )
                        nc.tensor.matmul(
                            out=ps[:],
                            lhsT=ht_t[:, j * P : (j + 1) * P],
                            rhs=wcat_t[:],
                            start=False,
                            stop=True,
                        )
                        dst = gs[:, (j - g0) * 2 * D : (j - g0 + 1) * 2 * D]
                        if ncopy % 2 == 0:
                            nc.vector.tensor_copy(out=dst, in_=ps[:])
                        else:
                            nc.scalar.copy(out=dst, in_=ps[:])
                        ncopy += 1
                    nc.sync.dma_start(
                        out=G[
                            (c0 + g0) * P : (c0 + g0 + gn) * P, :
                        ].rearrange("(j p) x -> p j x", p=P),
                        in_=gs[:, : gn * 2 * D].rearrange(
                            "p (j x) -> p j x", j=gn
                        ),
                    )

            # ---- phase 1b: Bh for local (permuted) nodes, straight into SBUF
            for blk in range(NBLK):
                ps = psB.tile([P, D], f32, tag="s")
                nc.tensor.matmul(
                    out=ps[:],
                    lhsT=ones_bf[:],
                    rhs=brhs_t[:, D : 2 * D],
                    start=True,
                    stop=False,
                )
                nc.tensor.matmul(
                    out=ps[:],
                    lhsT=hTloc_sb[:, blk * P : (blk + 1) * P],
                    rhs=wcat_t[:, D : 2 * D],
                    start=False,
                    stop=True,
                )
                nc.vector.tensor_copy(
                    out=bhloc_sb[:, blk * D : (blk + 1) * D], in_=ps[:]
                )

            # collapse phase-1 -> phase-2 deps into one barrier (sync-wait
            # slots on a single instruction are limited)
            tc.strict_bb_all_engine_barrier()

            # ---- phase 2+3 per gather group
            ob_t = None
            for (b0, n, kg, goff) in groups:
                gt = gatpool.tile([P, n * kg * 2 * D], bf16, tag="gt")
                nc.gpsimd.indirect_dma_start(
                    out=gt[:],
                    out_offset=None,
                    in_=G[:, :],
                    in_offset=bass.IndirectOffsetOnAxis(
                        ap=idx_t[:, goff : goff + n * kg], axis=0
                    ),
                )

                v4 = gt[:].rearrange("p (n k x) -> p n k x", n=n, x=2 * D)
                ah = v4[:, :, :, 0:D]
                bh = v4[:, :, :, D : 2 * D]
                bhd = (
                    bhloc_sb[:, b0 * D : (b0 + n) * D]
                    .rearrange("p (n o x) -> p n o x", n=n, o=1)
                    .to_broadcast([P, n, kg, D])
                )

                # gate = sigmoid(Bh[src] + Bh[dst]); message = gate * Ah[src]
                nc.vector.tensor_tensor(out=bh, in0=bh, in1=bhd, op=ad)
                nc.scalar.activation(out=bh, in_=bh, func=Sig)
                nc.gpsimd.tensor_tensor(out=bh, in0=bh, in1=ah, op=ml)

                # segment max: in-place overlapped-halving tree along k, then a
                # final level with the 0-clamp fused (max(a, 0) max b)
                k = kg
                while k > 2:
                    hh = (k + 1) // 2
                    nc.vector.tensor_tensor(
                        out=bh[:, :, 0:hh, :],
                        in0=bh[:, :, 0:hh, :],
                        in1=bh[:, :, k - hh : k, :],
                        op=mx,
                    )
                    k = hh
                cb_view = cball[:, b0 * D : (b0 + n) * D].rearrange(
                    "p (n o x) -> p n o x", n=n, o=1
                )
                nc.vector.tensor_tensor(
                    out=cb_view,
                    in0=bh[:, :, 0:1, :],
                    in1=bh[:, :, 1:2, :],
                    op=mx,
                )
                # clamp to 0 (covers no-in-edge nodes and all-negative maxes)
                nc.vector.tensor_scalar_max(
                    cball[:, b0 * D : (b0 + n) * D],
                    cball[:, b0 * D : (b0 + n) * D],
                    0.0,
                )

                # ---- phase 3 per block: bundle = h @ U1 + c @ U2 + U_b (bf16)
                for j in range(n):
                    blk = b0 + j
                    ctps = psB.tile([P, D], bf16, tag="t")
                    nc.tensor.transpose(
                        out=ctps[:],
                        in_=cball[:, blk * D : (blk + 1) * D],
                        identity=ident[:],
                    )
                    ct = scpool.tile([P, D], bf16, tag="ct")
                    nc.scalar.copy(out=ct[:], in_=ctps[:])

                    bp = psB.tile([P, D], f32, tag="s")
                    nc.tensor.matmul(
                        out=bp[:], lhsT=ones_bf[:], rhs=ubr_t[:],
                        start=True, stop=False,
                    )
                    nc.tensor.matmul(
                        out=bp[:],
                        lhsT=hTloc_sb[:, blk * P : (blk + 1) * P],
                        rhs=u1_t[:],
                        start=False,
                        stop=False,
                    )
                    nc.tensor.matmul(
                        out=bp[:], lhsT=ct[:], rhs=u2_t[:],
                        start=False, stop=True,
                    )

                    sq = scpool.tile([P, D], bf16, tag="sq")
                    nc.scalar.activation(
                        out=sq[:],
                        in_=bp[:],
                        func=mybir.ActivationFunctionType.Square,
                        accum_out=ssqall[:, blk : blk + 1],
                    )
                    nc.scalar.activation(
                        out=bpall[:, blk * D : (blk + 1) * D], in_=bp[:],
                        func=Relu,
                    )

            # ---- tail: batched norm + scale + residual + output
            nc.scalar.sqrt(nrm_t[:], ssqall[:])
            nc.vector.tensor_scalar_max(nrm_t[:], nrm_t[:], EPS)
            nc.vector.reciprocal(rn_t[:], nrm_t[:])

            for o0 in range(0, NBLK, OBW):
                on = min(OBW, NBLK - o0)
                ob_t = obpool.tile([P, OBW * D], f32, tag="ob")
                for j in range(on):
                    blk = o0 + j
                    nc.vector.tensor_scalar_mul(
                        ob_t[:, j * D : (j + 1) * D],
                        bpall[:, blk * D : (blk + 1) * D],
                        rn_t[:, blk : blk + 1],
                    )
                    nc.vector.tensor_tensor(
                        out=ob_t[:, j * D : (j + 1) * D],
                        in0=ob_t[:, j * D : (j + 1) * D],
                        in1=hloc_sb[:, blk * D : (blk + 1) * D],
                        op=ad,
                    )
                nc.sync.dma_start(
                    out=outp[o0 * P : (o0 + on) * P, :].rearrange(
                        "(j p) x -> p j x", p=P
                    ),
                    in_=ob_t[:, : on * D].rearrange("p (j x) -> p j x", j=on),
                )

    nc.compile()
    return nc


# --------------------------------------------------------------------------- run


def _run(inputs, trace=False):
    h = np.asarray(inputs["h"], np.float32)
    A_w = np.asarray(inputs["A_w"], np.float32)
    A_b = np.asarray(inputs["A_b"], np.float32)
    B_w = np.asarray(inputs["B_w"], np.float32)
    B_b = np.asarray(inputs["B_b"], np.float32)
    U_w = np.asarray(inputs["U_w"], np.float32)
    U_b = np.asarray(inputs["U_b"], np.float32)

    groups, total_cols, per_core = _preprocess(h, inputs["src"], inputs["dst"])

    key = (tuple(groups), total_cols)
    if key not in _prog_cache:
        _prog_cache.clear()
        _prog_cache[key] = _build(groups, total_cols)
    nc = _prog_cache[key]

    hT_bf = np.zeros((D, NPADG), BF16)
    hT_bf[:, :N] = h.T.astype(BF16)
    wcat = np.concatenate([A_w, B_w], axis=1).astype(BF16)
    brhs = np.concatenate([A_b, B_b])[None, :].astype(BF16)
    u1 = np.ascontiguousarray(U_w[:D]).astype(BF16)
    u2 = np.ascontiguousarray(U_w[D:]).astype(BF16)
    ubr = U_b[None, :].astype(BF16)

    in_maps = []
    for c in range(NCORES):
        gidx_c, perm = per_core[c]
        hl = np.zeros((NLOCP, D), np.float32)
        hl[:NLOC] = h[c * NLOC + perm]
        in_maps.append(
            {
                "hT": hT_bf,
                "hloc": hl,
                "hTloc": np.ascontiguousarray(hl.T).astype(BF16),
                "gidx": gidx_c,
                "wcat": wcat,
                "brhs": brhs,
                "u1": u1,
                "u2": u2,
                "ubr": ubr,
            }
        )

    res = run_bass_kernel_spmd(nc, in_maps, list(range(NCORES)), trace=trace)

    out = np.empty((N, D), np.float32)
    for c in range(NCORES):
        _, perm = per_core[c]
        out[c * NLOC + perm] = res.results[c]["out"][:NLOC]
    return out, res


def kernel(**inputs) -> np.ndarray:
    out, _ = _run(inputs, trace=False)
    return out


# revision 15
# speedup vs baseline: 1.2138x; 1.2138x over previous
"""GraphSAGE edge layer (sigmoid-gated message passing + segment-max) on 8 Trainium2
NeuronCores via Bass/Tile.

Strategy (graph/data parallel):
  - Nodes are sharded across 8 cores (6250 each); edges partitioned by destination
    node so the segment-max reduce is core-local.
  - Per core, a replicated G table [N, 256] = [Ah | Bh] (bf16) is built on-device
    (node-major rows so the per-edge indirect gather reads one 512B row per slot).
    The G build runs 2 node-blocks per PSUM bank with one wide bias matmul;
    PSUM->SBUF copies alternate between Vector and Scalar per write group so
    neither engine gates the PE.
  - Edges are packed host-side into a degree-bucketed (node x slot) grid with
    uniform slots-per-node within each gather group (1-4 blocks of 128 nodes).
    Pad slots point at a zero row of G, whose Ah half is 0 => gated contribution
    is exactly 0, absorbed by the final clamp.
  - Bh[dst] for local nodes is computed directly into SBUF (no gather needed).
  - Per group: ONE indirect DMA gathers all slot rows; the gate is computed
    in-place in the gathered tile (add/mult on Vector, sigmoid on Scalar); the
    segment max runs as a contiguous in-place overlapped-halving max tree.
    A few groups run their elementwise chain on GpSimd to offload Vector.
  - Update MLP runs per block in bf16 (c^T via one PE transpose); the small
    per-block tail ops are batched per group, and sqrt/normalize is deferred to
    one batched tail pass so the Scalar engine never swaps activation tables.
"""

import numpy as np
import ml_dtypes

from concourse import bass, bacc, mybir
from concourse.tile import TileContext
from concourse.bass_utils import run_bass_kernel_spmd
from concourse.masks import make_identity

BF16 = ml_dtypes.bfloat16

N = 50000
E = 800000
D = 128
NCORES = 8
NLOC = N // NCORES          # 6250 nodes per core
P = 128
NBLK1 = (N + P - 1) // P    # 391 G-build blocks
NPADG = NBLK1 * P           # 50048 (pad/zero row of G lives at this index)
NBLK = (NLOC + P - 1) // P  # 49 local node blocks
NLOCP = NBLK * P            # 6272
EPS = 1e-12
CH1 = 32                    # G-build hT chunk (blocks per DMA)
GW1 = 16                    # G-build write chunk (blocks per DMA)
GCAP = 48                   # max slots per gather group (48*512B = 24KB/partition)
OBW = 8                     # output write batch (blocks)
POOL_EVERY = 5              # every POOL_EVERY-th gather group runs on GpSimd

_prog_cache = {}


# --------------------------------------------------------------------------- host


def _preprocess(h, src, dst):
    """Shard edges by destination, build per-core degree-sorted slot grids.

    Blocks are grouped (1-4 per gather) with a uniform slot count kg per group.
    Returns (groups, total_cols, per_core) where groups[i] = (b0, n, kg, off)
    and per_core[c] = (gidx [128, total_cols] int32, perm [NLOC] int64).
    """
    src = np.asarray(src).astype(np.int64)
    dst = np.asarray(dst).astype(np.int64)

    order = np.argsort(dst, kind="stable")
    dst_s = dst[order]
    src_s = src[order]
    bounds = np.searchsorted(dst_s, np.arange(NCORES + 1) * NLOC)

    cores = []
    blkmax = np.zeros((NCORES, NBLK), np.int64)
    for c in range(NCORES):
        a, b = bounds[c], bounds[c + 1]
        ldst = dst_s[a:b] - c * NLOC
        lsrc = src_s[a:b]
        deg = np.bincount(ldst, minlength=NLOC)
        perm = np.argsort(-deg, kind="stable")
        pdeg = np.zeros(NLOCP, np.int64)
        pdeg[:NLOC] = deg[perm]
        blkmax[c] = pdeg.reshape(NBLK, P).max(axis=1)
        cores.append((lsrc, deg, perm))

    K = blkmax.max(axis=0)
    K = np.maximum((K + 1) // 2 * 2, 2).astype(np.int64)  # even, >= 2

    groups = []
    b = 0
    off = 0
    while b < NBLK:
        kg = int(K[b])
        n = max(1, min(4, NBLK - b, GCAP // kg))
        groups.append((b, n, kg, off))
        off += n * kg
        b += n
    total_cols = int(off)

    col0 = np.zeros(NBLK, np.int64)
    for (b0, n, kg, goff) in groups:
        for j in range(n):
            col0[b0 + j] = goff + j * kg

    per_core = []
    for c in range(NCORES):
        lsrc, deg, perm = cores[c]
        start = np.concatenate([[0], np.cumsum(deg)])
        gidx = np.full((P, total_cols), NPADG, np.int32)
        for blk in range(NBLK):
            o = int(col0[blk])
            for p in range(P):
                i = blk * P + p
                if i >= NLOC:
                    continue
                node = int(perm[i])
                dn = int(deg[node])
                if dn:
                    gidx[p, o : o + dn] = lsrc[start[node] : start[node] + dn]
        per_core.append((gidx, perm))
    return groups, total_cols, per_core


# --------------------------------------------------------------------------- bass


def _build(groups, total_cols):
    f32 = mybir.dt.float32
    bf16 = mybir.dt.bfloat16
    i32 = mybir.dt.int32
    Sig = mybir.ActivationFunctionType.Sigmoid
    Relu = mybir.ActivationFunctionType.Relu
    mx = mybir.AluOpType.max
    ad = mybir.AluOpType.add
    ml = mybir.AluOpType.mult

    nc = bacc.Bacc(
        "TRN2", target_bir_lowering=False, debug=False, num_devices=NCORES
    )
    hT = nc.declare_dram_parameter("hT", [D, NPADG], bf16, isOutput=False)
    hloc = nc.declare_dram_parameter("hloc", [NLOCP, D], f32, isOutput=False)
    hTloc = nc.declare_dram_parameter("hTloc", [D, NLOCP], bf16, isOutput=False)
    gidx = nc.declare_dram_parameter("gidx", [P, total_cols], i32, isOutput=False)
    wcat = nc.declare_dram_parameter("wcat", [D, 2 * D], bf16, isOutput=False)
    brhsw = nc.declare_dram_parameter("brhsw", [1, 4 * D], bf16, isOutput=False)
    brhsb = nc.declare_dram_parameter("brhsb", [1, 4 * D], bf16, isOutput=False)
    u1 = nc.declare_dram_parameter("u1", [D, D], bf16, isOutput=False)
    u2 = nc.declare_dram_parameter("u2", [D, D], bf16, isOutput=False)
    ubr = nc.declare_dram_parameter("ubr", [1, D], bf16, isOutput=False)
    outp = nc.declare_dram_parameter("out", [NLOCP, D], f32, isOutput=True)

    G = nc.dram_tensor("G", [NPADG + 1, 2 * D], bf16)

    with TileContext(nc) as tc:
        with (
            tc.tile_pool(name="const", bufs=1) as cpool,
            tc.tile_pool(name="ht1", bufs=2) as htpool,
            tc.tile_pool(name="gs1", bufs=2) as gspool,
            tc.tile_pool(name="gat", bufs=2) as gatpool,
            tc.tile_pool(name="sc", bufs=2) as scpool,
            tc.tile_pool(name="ob", bufs=2) as obpool,
            tc.tile_pool(name="psA", bufs=4, space="PSUM") as psA,
            tc.tile_pool(name="psB", bufs=2, space="PSUM") as psB,
            tc.tile_pool(name="psC", bufs=2, space="PSUM") as psC,
        ):
            # ---- constants
            wcat_t = cpool.tile([D, 2 * D], bf16)
            nc.sync.dma_start(out=wcat_t[:], in_=wcat[:, :])
            brhsw_t = cpool.tile([1, 4 * D], bf16)
            nc.sync.dma_start(out=brhsw_t[:], in_=brhsw[:, :])
            brhsb_t = cpool.tile([1, 4 * D], bf16)
            nc.sync.dma_start(out=brhsb_t[:], in_=brhsb[:, :])
            u1_t = cpool.tile([D, D], bf16)
            nc.sync.dma_start(out=u1_t[:], in_=u1[:, :])
            u2_t = cpool.tile([D, D], bf16)
            nc.sync.dma_start(out=u2_t[:], in_=u2[:, :])
            ubr_t = cpool.tile([1, D], bf16)
            nc.sync.dma_start(out=ubr_t[:], in_=ubr[:, :])
            idx_t = cpool.tile([P, total_cols], i32)
            nc.sync.dma_start(out=idx_t[:], in_=gidx[:, :])

            ones_bf = cpool.tile([1, P], bf16)
            nc.vector.memset(ones_bf[:], 1.0)
            ident = cpool.tile([P, P], bf16)
            make_identity(nc, ident[:])

            zrow = cpool.tile([1, 2 * D], bf16)
            nc.vector.memset(zrow[:], 0.0)
            nc.sync.dma_start(out=G[NPADG : NPADG + 1, :], in_=zrow[:])

            # ---- preload per-core local h (both layouts) once
            hloc_sb = cpool.tile([P, NBLK * D], f32)
            nc.sync.dma_start(
                out=hloc_sb[:].rearrange("p (b x) -> p b x", b=NBLK),
                in_=hloc[:, :].rearrange("(b p) x -> p b x", p=P),
            )
            hTloc_sb = cpool.tile([D, NLOCP], bf16)
            nc.sync.dma_start(out=hTloc_sb[:], in_=hTloc[:, :])

            bhloc_sb = cpool.tile([P, NBLK * D], bf16)
            cball = cpool.tile([P, NBLK * D], bf16)
            bpall = cpool.tile([P, NBLK * D], f32)
            ssqall = cpool.tile([P, NBLK], f32)
            nrm_t = cpool.tile([P, NBLK], f32)
            rn_t = cpool.tile([P, NBLK], f32)

            # ---- phase 1: G = [h @ A_w + A_b | h @ B_w + B_b]  (bf16)
            # Two node blocks share one PSUM bank; one wide bias matmul seeds
            # both; copies alternate Vector/Scalar per write group.
            gsgrp = 0
            for c0 in range(0, NBLK1, CH1):
                nb = min(CH1, NBLK1 - c0)
                ht_t = htpool.tile([D, CH1 * P], bf16, tag="ht")
                nc.sync.dma_start(
                    out=ht_t[:, : nb * P],
                    in_=hT[:, c0 * P : (c0 + nb) * P],
                )
                for g0 in range(0, nb, GW1):
                    gn = min(GW1, nb - g0)
                    gs = gspool.tile([P, GW1 * 2 * D], bf16, tag="gs")
                    eng = nc.vector if gsgrp % 2 == 0 else nc.scalar
                    gsgrp += 1
                    for j0 in range(g0, g0 + gn, 2):
                        jn = min(2, g0 + gn - j0)
                        ps = psA.tile([P, 4 * D], f32, tag="g")
                        nc.tensor.matmul(
                            out=ps[:, : jn * 2 * D],
                            lhsT=ones_bf[:],
                            rhs=brhsw_t[:, : jn * 2 * D],
                            start=True,
                            stop=False,
                        )
                        for j in range(j0, j0 + jn):
                            nc.tensor.matmul(
                                out=ps[:, (j - j0) * 2 * D : (j - j0 + 1) * 2 * D],
                                lhsT=ht_t[:, j * P : (j + 1) * P],
                                rhs=wcat_t[:],
                                start=False,
                                stop=True,
                            )
                        if eng is nc.vector:
                            nc.vector.tensor_copy(
                                out=gs[
                                    :,
                                    (j0 - g0) * 2 * D : (j0 - g0 + jn) * 2 * D,
                                ],
                                in_=ps[:, : jn * 2 * D],
                            )
                        else:
                            nc.scalar.copy(
                                out=gs[
                                    :,
                                    (j0 - g0) * 2 * D : (j0 - g0 + jn) * 2 * D,
                                ],
                                in_=ps[:, : jn * 2 * D],
                            )
                    nc.sync.dma_start(
                        out=G[
                            (c0 + g0) * P : (c0 + g0 + gn) * P, :
                        ].rearrange("(j p) x -> p j x", p=P),
                        in_=gs[:, : gn * 2 * D].rearrange(
                            "p (j x) -> p j x", j=gn
                        ),
                    )

            # ---- phase 1b: Bh for local (permuted) nodes, straight into SBUF
            for b0 in range(0, NBLK, 4):
                bn = min(4, NBLK - b0)
                ps = psB.tile([P, 4 * D], f32, tag="s")
                nc.tensor.matmul(
                    out=ps[:, : bn * D],
                    lhsT=ones_bf[:],
                    rhs=brhsb_t[:, : bn * D],
                    start=True,
                    stop=False,
                )
                for j in range(bn):
                    blk = b0 + j
                    nc.tensor.matmul(
                        out=ps[:, j * D : (j + 1) * D],
                        lhsT=hTloc_sb[:, blk * P : (blk + 1) * P],
                        rhs=wcat_t[:, D : 2 * D],
                        start=False,
                        stop=True,
                    )
                nc.vector.tensor_copy(
                    out=bhloc_sb[:, b0 * D : (b0 + bn) * D], in_=ps[:, : bn * D]
                )

            # collapse phase-1 -> phase-2 deps into one barrier (sync-wait
            # slots on a single instruction are limited)
            tc.strict_bb_all_engine_barrier()

            # ---- phase 2+3 per gather group
            for gi, (b0, n, kg, goff) in enumerate(groups):
                gt = gatpool.tile([P, n * kg * 2 * D], bf16, tag="gt")
                nc.gpsimd.indirect_dma_start(
                    out=gt[:],
                    out_offset=None,
                    in_=G[:, :],
                    in_offset=bass.IndirectOffsetOnAxis(
                        ap=idx_t[:, goff : goff + n * kg], axis=0
                    ),
                )

                # gating mult runs on GpSimd for a few groups to offload Vector
                ve = nc.gpsimd if gi % POOL_EVERY == POOL_EVERY - 1 else nc.vector

                v4 = gt[:].rearrange("p (n k x) -> p n k x", n=n, x=2 * D)
                ah = v4[:, :, :, 0:D]
                bh = v4[:, :, :, D : 2 * D]
                bhd = (
                    bhloc_sb[:, b0 * D : (b0 + n) * D]
                    .rearrange("p (n o x) -> p n o x", n=n, o=1)
                    .to_broadcast([P, n, kg, D])
                )

                # gate = sigmoid(Bh[src] + Bh[dst]); message = gate * Ah[src]
                nc.vector.tensor_tensor(out=bh, in0=bh, in1=bhd, op=ad)
                nc.scalar.activation(out=bh, in_=bh, func=Sig)
                ve.tensor_tensor(out=bh, in0=bh, in1=ah, op=ml)

                # segment max: in-place overlapped-halving tree along k
                k = kg
                while k > 2:
                    hh = (k + 1) // 2
                    nc.vector.tensor_tensor(
                        out=bh[:, :, 0:hh, :],
                        in0=bh[:, :, 0:hh, :],
                        in1=bh[:, :, k - hh : k, :],
                        op=mx,
                    )
                    k = hh
                cb_view = cball[:, b0 * D : (b0 + n) * D].rearrange(
                    "p (n o x) -> p n o x", n=n, o=1
                )
                nc.vector.tensor_tensor(
                    out=cb_view,
                    in0=bh[:, :, 0:1, :],
                    in1=bh[:, :, 1:2, :],
                    op=mx,
                )
                # clamp to 0 (covers no-in-edge nodes and all-negative maxes)
                nc.vector.tensor_scalar_max(
                    cball[:, b0 * D : (b0 + n) * D],
                    cball[:, b0 * D : (b0 + n) * D],
                    0.0,
                )

                # ---- phase 3, batched per group: bundle = h@U1 + c@U2 + U_b
                ctps = psC.tile([P, 4 * D], bf16, tag="t")
                for j in range(n):
                    blk = b0 + j
                    nc.tensor.transpose(
                        out=ctps[:, j * D : (j + 1) * D],
                        in_=cball[:, blk * D : (blk + 1) * D],
                        identity=ident[:],
                    )
                ct = scpool.tile([P, 4 * D], bf16, tag="ct")
                nc.vector.tensor_copy(out=ct[:, : n * D], in_=ctps[:, : n * D])

                bp = psB.tile([P, 4 * D], f32, tag="s")
                for j in range(n):
                    blk = b0 + j
                    sl = slice(j * D, (j + 1) * D)
                    nc.tensor.matmul(
                        out=bp[:, sl], lhsT=ones_bf[:], rhs=ubr_t[:],
                        start=True, stop=False,
                    )
                    nc.tensor.matmul(
                        out=bp[:, sl],
                        lhsT=hTloc_sb[:, blk * P : (blk + 1) * P],
                        rhs=u1_t[:],
                        start=False,
                        stop=False,
                    )
                    nc.tensor.matmul(
                        out=bp[:, sl], lhsT=ct[:, sl], rhs=u2_t[:],
                        start=False, stop=True,
                    )

                sq = scpool.tile([P, 4 * D], bf16, tag="sq")
                nc.scalar.activation(
                    out=sq[:, : n * D], in_=bp[:, : n * D],
                    func=mybir.ActivationFunctionType.Square,
                )
                nc.vector.tensor_reduce(
                    out=ssqall[:, b0 : b0 + n],
                    in_=sq[:, : n * D].rearrange("p (n x) -> p n x", n=n),
                    axis=mybir.AxisListType.X,
                    op=ad,
                )
                nc.scalar.activation(
                    out=bpall[:, b0 * D : (b0 + n) * D], in_=bp[:, : n * D],
                    func=Relu,
                )

            # ---- tail: batched norm + scale + residual + output
            nc.scalar.sqrt(nrm_t[:], ssqall[:])
            nc.vector.tensor_scalar_max(nrm_t[:], nrm_t[:], EPS)
            nc.vector.reciprocal(rn_t[:], nrm_t[:])

            for o0 in range(0, NBLK, OBW):
                on = min(OBW, NBLK - o0)
                ob_t = obpool.tile([P, OBW * D], f32, tag="ob")
                for j in range(on):
                    blk = o0 + j
                    nc.vector.tensor_scalar_mul(
                        ob_t[:, j * D : (j + 1) * D],
                        bpall[:, blk * D : (blk + 1) * D],
                        rn_t[:, blk : blk + 1],
                    )
                nc.vector.tensor_tensor(
                    out=ob_t[:, : on * D],
                    in0=ob_t[:, : on * D],
                    in1=hloc_sb[:, o0 * D : (o0 + on) * D],
                    op=ad,
                )
                nc.sync.dma_start(
                    out=outp[o0 * P : (o0 + on) * P, :].rearrange(
                        "(j p) x -> p j x", p=P
                    ),
                    in_=ob_t[:, : on * D].rearrange("p (j x) -> p j x", j=on),
                )

    nc.compile()
    return nc


# --------------------------------------------------------------------------- run


def _run(inputs, trace=False):
    h = np.asarray(inputs["h"], np.float32)
    A_w = np.asarray(inputs["A_w"], np.float32)
    A_b = np.asarray(inputs["A_b"], np.float32)
    B_w = np.asarray(inputs["B_w"], np.float32)
    B_b = np.asarray(inputs["B_b"], np.float32)
    U_w = np.asarray(inputs["U_w"], np.float32)
    U_b = np.asarray(inputs["U_b"], np.float32)

    groups, total_cols, per_core = _preprocess(h, inputs["src"], inputs["dst"])

    key = (tuple(groups), total_cols)
    if key not in _prog_cache:
        _prog_cache.clear()
        _prog_cache[key] = _build(groups, total_cols)
    nc = _prog_cache[key]

    hT_bf = np.zeros((D, NPADG), BF16)
    hT_bf[:, :N] = h.T.astype(BF16)
    wcat = np.concatenate([A_w, B_w], axis=1).astype(BF16)
    brhs = np.concatenate([A_b, B_b])
    brhsw = np.concatenate([brhs, brhs])[None, :].astype(BF16)
    brhsb = np.concatenate([B_b] * 4)[None, :].astype(BF16)
    u1 = np.ascontiguousarray(U_w[:D]).astype(BF16)
    u2 = np.ascontiguousarray(U_w[D:]).astype(BF16)
    ubr = U_b[None, :].astype(BF16)

    in_maps = []
    for c in range(NCORES):
        gidx_c, perm = per_core[c]
        hl = np.zeros((NLOCP, D), np.float32)
        hl[:NLOC] = h[c * NLOC + perm]
        in_maps.append(
            {
                "hT": hT_bf,
                "hloc": hl,
                "hTloc": np.ascontiguousarray(hl.T).astype(BF16),
                "gidx": gidx_c,
                "wcat": wcat,
                "brhsw": brhsw,
                "brhsb": brhsb,
                "u1": u1,
                "u2": u2,
                "ubr": ubr,
            }
        )

    res = run_bass_kernel_spmd(nc, in_maps, list(range(NCORES)), trace=trace)

    out = np.empty((N, D), np.float32)
    for c in range(NCORES):
        _, perm = per_core[c]
        out[c * NLOC + perm] = res.results[c]["out"][:NLOC]
    return out, res


def kernel(**inputs) -> np.ndarray:
    out, _ = _run(inputs, trace=False)
    return out


# revision 16
# speedup vs baseline: 1.5180x; 1.2507x over previous
"""GraphSAGE edge layer (sigmoid-gated message passing + segment-max) on 8 Trainium2
NeuronCores via Bass/Tile.

Strategy (graph/data parallel):
  - Nodes are sharded across 8 cores (6250 each); edges partitioned by destination
    node so the segment-max reduce is core-local.
  - Per core, a replicated G table [N, 256] = [Ah | Bh] (bf16) is built on-device
    (node-major rows so the per-edge indirect gather reads one 512B row per slot).
    The G build runs 2 node-blocks per PSUM bank with one wide bias matmul;
    PSUM->SBUF copies alternate between Vector and Scalar per write group so
    neither engine gates the PE.
  - Edges are packed host-side into a degree-bucketed (node x slot) grid with
    uniform slots-per-node within each gather group (1-4 blocks of 128 nodes).
    Pad slots point at a zero row of G, whose Ah half is 0 => gated contribution
    is exactly 0, absorbed by the final clamp.
  - Bh[dst] for local nodes is computed directly into SBUF (no gather needed).
  - Per group: ONE indirect DMA gathers all slot rows; the gate is computed
    in-place in the gathered tile (add/mult on Vector, sigmoid on Scalar); the
    segment max runs as a contiguous in-place overlapped-halving max tree.
    A few groups run their elementwise chain on GpSimd to offload Vector.
  - Update MLP runs per block in bf16 (c^T via one PE transpose); the small
    per-block tail ops are batched per group, and sqrt/normalize is deferred to
    one batched tail pass so the Scalar engine never swaps activation tables.
"""

import numpy as np
import ml_dtypes

from concourse import bass, bacc, mybir
from concourse.tile import TileContext
from concourse.bass_utils import run_bass_kernel_spmd
from concourse.masks import make_identity

BF16 = ml_dtypes.bfloat16

N = 50000
E = 800000
D = 128
NCORES = 8
NLOC = N // NCORES          # 6250 nodes per core
P = 128
NBLK1 = (N + P - 1) // P    # 391 G-build blocks
NPADG = NBLK1 * P           # 50048 (pad/zero row of G lives at this index)
NBLK = (NLOC + P - 1) // P  # 49 local node blocks
NLOCP = NBLK * P            # 6272
EPS = 1e-12
CH1 = 16                    # G-build hT chunk (blocks per DMA)
GW1 = 8                     # G-build write chunk (blocks per DMA)
GCAP = 48                   # max slots per gather group (48*512B = 24KB/partition)
OBW = 8                     # output write batch (blocks)
POOL_EVERY = 5              # every POOL_EVERY-th gather group runs on GpSimd

_prog_cache = {}


# --------------------------------------------------------------------------- host


def _preprocess(h, src, dst):
    """Shard edges by destination, build per-core degree-sorted slot grids.

    Blocks are grouped (1-4 per gather) with a uniform slot count kg per group.
    Returns (groups, total_cols, per_core) where groups[i] = (b0, n, kg, off)
    and per_core[c] = (gidx [128, total_cols] int32, perm [NLOC] int64).
    """
    src = np.asarray(src).astype(np.int64)
    dst = np.asarray(dst).astype(np.int64)

    order = np.argsort(dst, kind="stable")
    dst_s = dst[order]
    src_s = src[order]
    bounds = np.searchsorted(dst_s, np.arange(NCORES + 1) * NLOC)

    cores = []
    blkmax = np.zeros((NCORES, NBLK), np.int64)
    for c in range(NCORES):
        a, b = bounds[c], bounds[c + 1]
        ldst = dst_s[a:b] - c * NLOC
        lsrc = src_s[a:b]
        deg = np.bincount(ldst, minlength=NLOC)
        perm = np.argsort(-deg, kind="stable")
        pdeg = np.zeros(NLOCP, np.int64)
        pdeg[:NLOC] = deg[perm]
        blkmax[c] = pdeg.reshape(NBLK, P).max(axis=1)
        cores.append((lsrc, deg, perm))

    K = blkmax.max(axis=0)
    K = np.maximum((K + 1) // 2 * 2, 2).astype(np.int64)  # even, >= 2

    groups = []
    b = 0
    off = 0
    while b < NBLK:
        kg = int(K[b])
        n = max(1, min(4, NBLK - b, GCAP // kg))
        groups.append((b, n, kg, off))
        off += n * kg
        b += n
    total_cols = int(off)

    col0 = np.zeros(NBLK, np.int64)
    for (b0, n, kg, goff) in groups:
        for j in range(n):
            col0[b0 + j] = goff + j * kg

    per_core = []
    for c in range(NCORES):
        lsrc, deg, perm = cores[c]
        start = np.concatenate([[0], np.cumsum(deg)])
        gidx = np.full((P, total_cols), NPADG, np.int32)
        for blk in range(NBLK):
            o = int(col0[blk])
            for p in range(P):
                i = blk * P + p
                if i >= NLOC:
                    continue
                node = int(perm[i])
                dn = int(deg[node])
                if dn:
                    gidx[p, o : o + dn] = lsrc[start[node] : start[node] + dn]
        per_core.append((gidx, perm))
    return groups, total_cols, per_core


# --------------------------------------------------------------------------- bass


def _build(groups, total_cols):
    f32 = mybir.dt.float32
    bf16 = mybir.dt.bfloat16
    i32 = mybir.dt.int32
    Sig = mybir.ActivationFunctionType.Sigmoid
    Relu = mybir.ActivationFunctionType.Relu
    mx = mybir.AluOpType.max
    ad = mybir.AluOpType.add
    ml = mybir.AluOpType.mult

    nc = bacc.Bacc(
        "TRN2", target_bir_lowering=False, debug=False, num_devices=NCORES
    )
    hT = nc.declare_dram_parameter("hT", [D, NPADG], bf16, isOutput=False)
    hloc = nc.declare_dram_parameter("hloc", [NLOCP, D], f32, isOutput=False)
    hTloc = nc.declare_dram_parameter("hTloc", [D, NLOCP], bf16, isOutput=False)
    gidx = nc.declare_dram_parameter("gidx", [P, total_cols], i32, isOutput=False)
    wcat = nc.declare_dram_parameter("wcat", [D, 2 * D], bf16, isOutput=False)
    brhsw = nc.declare_dram_parameter("brhsw", [1, 4 * D], bf16, isOutput=False)
    u1 = nc.declare_dram_parameter("u1", [D, D], bf16, isOutput=False)
    u2 = nc.declare_dram_parameter("u2", [D, D], bf16, isOutput=False)
    ubr = nc.declare_dram_parameter("ubr", [1, D], bf16, isOutput=False)
    outp = nc.declare_dram_parameter("out", [NLOCP, D], f32, isOutput=True)

    G = nc.dram_tensor("G", [NPADG + 1, 2 * D], bf16)

    with TileContext(nc) as tc:
        with (
            tc.tile_pool(name="const", bufs=1) as cpool,
            tc.tile_pool(name="ht1", bufs=2) as htpool,
            tc.tile_pool(name="gs1", bufs=2) as gspool,
            tc.tile_pool(name="gat", bufs=3) as gatpool,
            tc.tile_pool(name="sc", bufs=2) as scpool,
            tc.tile_pool(name="ob", bufs=2) as obpool,
            tc.tile_pool(name="psA", bufs=4, space="PSUM") as psA,
            tc.tile_pool(name="psB", bufs=2, space="PSUM") as psB,
            tc.tile_pool(name="psC", bufs=2, space="PSUM") as psC,
        ):
            # ---- constants
            wcat_t = cpool.tile([D, 2 * D], bf16)
            nc.sync.dma_start(out=wcat_t[:], in_=wcat[:, :])
            brhsw_t = cpool.tile([1, 4 * D], bf16)
            nc.sync.dma_start(out=brhsw_t[:], in_=brhsw[:, :])
            u1_t = cpool.tile([D, D], bf16)
            nc.sync.dma_start(out=u1_t[:], in_=u1[:, :])
            u2_t = cpool.tile([D, D], bf16)
            nc.sync.dma_start(out=u2_t[:], in_=u2[:, :])
            ubr_t = cpool.tile([1, D], bf16)
            nc.sync.dma_start(out=ubr_t[:], in_=ubr[:, :])
            idx_t = cpool.tile([P, total_cols], i32)
            nc.sync.dma_start(out=idx_t[:], in_=gidx[:, :])

            ones_bf = cpool.tile([1, P], bf16)
            nc.vector.memset(ones_bf[:], 1.0)
            ident = cpool.tile([P, P], bf16)
            make_identity(nc, ident[:])

            zrow = cpool.tile([1, 2 * D], bf16)
            nc.vector.memset(zrow[:], 0.0)
            nc.sync.dma_start(out=G[NPADG : NPADG + 1, :], in_=zrow[:])

            # replicated bias tile [A_b | 2*B_b | A_b | 2*B_b] (every partition)
            bias_sb = cpool.tile([P, 4 * D], f32)
            ps0 = psA.tile([P, 4 * D], f32, tag="g")
            nc.tensor.matmul(
                out=ps0[:], lhsT=ones_bf[:], rhs=brhsw_t[:],
                start=True, stop=True,
            )
            nc.vector.tensor_copy(out=bias_sb[:], in_=ps0[:])

            # ---- preload per-core local h (both layouts) once
            hloc_sb = cpool.tile([P, NBLK * D], f32)
            nc.sync.dma_start(
                out=hloc_sb[:].rearrange("p (b x) -> p b x", b=NBLK),
                in_=hloc[:, :].rearrange("(b p) x -> p b x", p=P),
            )
            hTloc_sb = cpool.tile([D, NLOCP], bf16)
            nc.sync.dma_start(out=hTloc_sb[:], in_=hTloc[:, :])

            bhloc_sb = cpool.tile([P, NBLK * D], bf16)
            cball = cpool.tile([P, NBLK * D], bf16)
            bpall = cpool.tile([P, NBLK * D], f32)
            ssqall = cpool.tile([P, NBLK], f32)
            nrm_t = cpool.tile([P, NBLK], f32)
            rn_t = cpool.tile([P, NBLK], f32)

            # ---- phase 1: G = [h @ A_w + A_b | h @ B_w + B_b]  (bf16)
            # Two node blocks share one PSUM bank; one wide bias matmul seeds
            # both; copies alternate Vector/Scalar per write group.
            gsgrp = 0
            for c0 in range(0, NBLK1, CH1):
                nb = min(CH1, NBLK1 - c0)
                ht_t = htpool.tile([D, CH1 * P], bf16, tag="ht")
                nc.sync.dma_start(
                    out=ht_t[:, : nb * P],
                    in_=hT[:, c0 * P : (c0 + nb) * P],
                )
                for g0 in range(0, nb, GW1):
                    gn = min(GW1, nb - g0)
                    gs = gspool.tile([P, GW1 * 2 * D], bf16, tag="gs")
                    for j0 in range(g0, g0 + gn, 2):
                        jn = min(2, g0 + gn - j0)
                        ps = psA.tile([P, 4 * D], f32, tag="g")
                        for j in range(j0, j0 + jn):
                            nc.tensor.matmul(
                                out=ps[:, (j - j0) * 2 * D : (j - j0 + 1) * 2 * D],
                                lhsT=ht_t[:, j * P : (j + 1) * P],
                                rhs=wcat_t[:],
                                start=True,
                                stop=True,
                            )
                        nc.vector.tensor_tensor(
                            out=gs[
                                :, (j0 - g0) * 2 * D : (j0 - g0 + jn) * 2 * D
                            ],
                            in0=ps[:, : jn * 2 * D],
                            in1=bias_sb[:, : jn * 2 * D],
                            op=mybir.AluOpType.add,
                        )
                    nc.sync.dma_start(
                        out=G[
                            (c0 + g0) * P : (c0 + g0 + gn) * P, :
                        ].rearrange("(j p) x -> p j x", p=P),
                        in_=gs[:, : gn * 2 * D].rearrange(
                            "p (j x) -> p j x", j=gn
                        ),
                    )

            # ---- phase 1b: Bh for local (permuted) nodes, straight into SBUF
            for b0 in range(0, NBLK, 4):
                bn = min(4, NBLK - b0)
                ps = psB.tile([P, 4 * D], f32, tag="s")
                for j in range(bn):
                    blk = b0 + j
                    nc.tensor.matmul(
                        out=ps[:, j * D : (j + 1) * D],
                        lhsT=hTloc_sb[:, blk * P : (blk + 1) * P],
                        rhs=wcat_t[:, D : 2 * D],
                        start=True,
                        stop=True,
                    )
                nc.scalar.copy(
                    out=bhloc_sb[:, b0 * D : (b0 + bn) * D], in_=ps[:, : bn * D]
                )

            # collapse phase-1 -> phase-2 deps into one barrier (sync-wait
            # slots on a single instruction are limited)
            tc.strict_bb_all_engine_barrier()

            # ---- phase 2+3 per gather group
            for gi, (b0, n, kg, goff) in enumerate(groups):
                gt = gatpool.tile([P, n * kg * 2 * D], bf16, tag="gt")
                nc.gpsimd.indirect_dma_start(
                    out=gt[:],
                    out_offset=None,
                    in_=G[:, :],
                    in_offset=bass.IndirectOffsetOnAxis(
                        ap=idx_t[:, goff : goff + n * kg], axis=0
                    ),
                )

                v4 = gt[:].rearrange("p (n k x) -> p n k x", n=n, x=2 * D)
                ah = v4[:, :, :, 0:D]
                bh = v4[:, :, :, D : 2 * D]
                bhd = (
                    bhloc_sb[:, b0 * D : (b0 + n) * D]
                    .rearrange("p (n o x) -> p n o x", n=n, o=1)
                    .to_broadcast([P, n, kg, D])
                )

                # gate = sigmoid(Bh[src] + Bh[dst]); message = gate * Ah[src]
                nc.vector.tensor_tensor(out=bh, in0=bh, in1=bhd, op=ad)
                nc.scalar.activation(out=bh, in_=bh, func=Sig)
                nc.vector.tensor_tensor(out=bh, in0=bh, in1=ah, op=ml)

                # segment max: in-place overlapped-halving tree along k
                k = kg
                while k > 2:
                    hh = (k + 1) // 2
                    nc.vector.tensor_tensor(
                        out=bh[:, :, 0:hh, :],
                        in0=bh[:, :, 0:hh, :],
                        in1=bh[:, :, k - hh : k, :],
                        op=mx,
                    )
                    k = hh
                cb_view = cball[:, b0 * D : (b0 + n) * D].rearrange(
                    "p (n o x) -> p n o x", n=n, o=1
                )
                nc.vector.tensor_tensor(
                    out=cb_view,
                    in0=bh[:, :, 0:1, :],
                    in1=bh[:, :, 1:2, :],
                    op=mx,
                )
                # clamp to 0 (covers no-in-edge nodes and all-negative maxes)
                nc.scalar.activation(
                    out=cball[:, b0 * D : (b0 + n) * D],
                    in_=cball[:, b0 * D : (b0 + n) * D],
                    func=Relu,
                )

                # ---- phase 3, batched per group: bundle = h@U1 + c@U2 + U_b
                ctps = psC.tile([P, 4 * D], bf16, tag="t")
                for j in range(n):
                    blk = b0 + j
                    nc.tensor.transpose(
                        out=ctps[:, j * D : (j + 1) * D],
                        in_=cball[:, blk * D : (blk + 1) * D],
                        identity=ident[:],
                    )
                ct = scpool.tile([P, 4 * D], bf16, tag="ct")
                nc.scalar.copy(out=ct[:, : n * D], in_=ctps[:, : n * D])

                bp = psB.tile([P, 4 * D], f32, tag="s")
                for j in range(n):
                    blk = b0 + j
                    sl = slice(j * D, (j + 1) * D)
                    nc.tensor.matmul(
                        out=bp[:, sl], lhsT=ones_bf[:], rhs=ubr_t[:],
                        start=True, stop=False,
                    )
                    nc.tensor.matmul(
                        out=bp[:, sl],
                        lhsT=hTloc_sb[:, blk * P : (blk + 1) * P],
                        rhs=u1_t[:],
                        start=False,
                        stop=False,
                    )
                    nc.tensor.matmul(
                        out=bp[:, sl], lhsT=ct[:, sl], rhs=u2_t[:],
                        start=False, stop=True,
                    )

                sq = scpool.tile([P, 4 * D], bf16, tag="sq")
                nc.scalar.activation(
                    out=sq[:, : n * D], in_=bp[:, : n * D],
                    func=mybir.ActivationFunctionType.Square,
                )
                nc.vector.tensor_reduce(
                    out=ssqall[:, b0 : b0 + n],
                    in_=sq[:, : n * D].rearrange("p (n x) -> p n x", n=n),
                    axis=mybir.AxisListType.X,
                    op=ad,
                )
                nc.scalar.activation(
                    out=bpall[:, b0 * D : (b0 + n) * D], in_=bp[:, : n * D],
                    func=Relu,
                )

            # ---- tail: batched norm + scale + residual + output
            nc.scalar.sqrt(nrm_t[:], ssqall[:])
            nc.vector.tensor_scalar_max(nrm_t[:], nrm_t[:], EPS)
            nc.vector.reciprocal(rn_t[:], nrm_t[:])

            for o0 in range(0, NBLK, OBW):
                on = min(OBW, NBLK - o0)
                ob_t = obpool.tile([P, OBW * D], f32, tag="ob")
                for j in range(on):
                    blk = o0 + j
                    nc.vector.tensor_scalar_mul(
                        ob_t[:, j * D : (j + 1) * D],
                        bpall[:, blk * D : (blk + 1) * D],
                        rn_t[:, blk : blk + 1],
                    )
                nc.vector.tensor_tensor(
                    out=ob_t[:, : on * D],
                    in0=ob_t[:, : on * D],
                    in1=hloc_sb[:, o0 * D : (o0 + on) * D],
                    op=ad,
                )
                nc.sync.dma_start(
                    out=outp[o0 * P : (o0 + on) * P, :].rearrange(
                        "(j p) x -> p j x", p=P
                    ),
                    in_=ob_t[:, : on * D].rearrange("p (j x) -> p j x", j=on),
                )

    nc.compile()
    return nc


# --------------------------------------------------------------------------- run


def _run(inputs, trace=False):
    h = np.asarray(inputs["h"], np.float32)
    A_w = np.asarray(inputs["A_w"], np.float32)
    A_b = np.asarray(inputs["A_b"], np.float32)
    B_w = np.asarray(inputs["B_w"], np.float32)
    B_b = np.asarray(inputs["B_b"], np.float32)
    U_w = np.asarray(inputs["U_w"], np.float32)
    U_b = np.asarray(inputs["U_b"], np.float32)

    groups, total_cols, per_core = _preprocess(h, inputs["src"], inputs["dst"])

    key = (tuple(groups), total_cols)
    if key not in _prog_cache:
        _prog_cache.clear()
        _prog_cache[key] = _build(groups, total_cols)
    nc = _prog_cache[key]

    hT_bf = np.zeros((D, NPADG), BF16)
    hT_bf[:, :N] = h.T.astype(BF16)
    wcat = np.concatenate([A_w, B_w], axis=1).astype(BF16)
    brhs = np.concatenate([A_b, 2.0 * B_b])
    brhsw = np.concatenate([brhs, brhs])[None, :].astype(BF16)
    u1 = np.ascontiguousarray(U_w[:D]).astype(BF16)
    u2 = np.ascontiguousarray(U_w[D:]).astype(BF16)
    ubr = U_b[None, :].astype(BF16)

    in_maps = []
    for c in range(NCORES):
        gidx_c, perm = per_core[c]
        hl = np.zeros((NLOCP, D), np.float32)
        hl[:NLOC] = h[c * NLOC + perm]
        in_maps.append(
            {
                "hT": hT_bf,
                "hloc": hl,
                "hTloc": np.ascontiguousarray(hl.T).astype(BF16),
                "gidx": gidx_c,
                "wcat": wcat,
                "brhsw": brhsw,
                "u1": u1,
                "u2": u2,
                "ubr": ubr,
            }
        )

    res = run_bass_kernel_spmd(nc, in_maps, list(range(NCORES)), trace=trace)

    out = np.empty((N, D), np.float32)
    for c in range(NCORES):
        _, perm = per_core[c]
        out[c * NLOC + perm] = res.results[c]["out"][:NLOC]
    return out, res


def kernel(**inputs) -> np.ndarray:
    out, _ = _run(inputs, trace=False)
    return out


# revision 21
# speedup vs baseline: 1.6273x; 1.0720x over previous
"""GraphSAGE edge layer (sigmoid-gated message passing + segment-max) on 8 Trainium2
NeuronCores via Bass/Tile.

Strategy (graph/data parallel):
  - Nodes are sharded across 8 cores (6250 each); edges partitioned by destination
    node so the segment-max reduce is core-local.
  - Per core, a replicated G table [N, 256] = [Ah | Bh] (bf16) is built on-device
    (node-major rows so the per-edge indirect gather reads one 512B row per slot).
    The G build runs 2 node-blocks per PSUM bank with one wide bias matmul;
    PSUM->SBUF copies alternate between Vector and Scalar per write group so
    neither engine gates the PE.
  - Edges are packed host-side into a degree-bucketed (node x slot) grid with
    uniform slots-per-node within each gather group (1-4 blocks of 128 nodes).
    Pad slots point at a zero row of G, whose Ah half is 0 => gated contribution
    is exactly 0, absorbed by the final clamp.
  - Bh[dst] for local nodes is computed directly into SBUF (no gather needed).
  - Per group: ONE indirect DMA gathers all slot rows; the gate is computed
    in-place in the gathered tile (add/mult on Vector, sigmoid on Scalar); the
    segment max runs as a contiguous in-place overlapped-halving max tree.
    A few groups run their elementwise chain on GpSimd to offload Vector.
  - Update MLP runs per block in bf16 (c^T via one PE transpose); the small
    per-block tail ops are batched per group, and sqrt/normalize is deferred to
    one batched tail pass so the Scalar engine never swaps activation tables.
"""

import numpy as np
import ml_dtypes

from concourse import bass, bacc, mybir
from concourse.tile import TileContext
from concourse.bass_utils import run_bass_kernel_spmd
from concourse.masks import make_identity

BF16 = ml_dtypes.bfloat16

N = 50000
E = 800000
D = 128
NCORES = 8
NLOC = N // NCORES          # 6250 nodes per core
P = 128
NBLK1 = (N + P - 1) // P    # 391 G-build blocks
NPADG = NBLK1 * P           # 50048 (pad/zero row of G lives at this index)
NBLK = (NLOC + P - 1) // P  # 49 local node blocks
NLOCP = NBLK * P            # 6272
EPS = 1e-12
CH1 = 16                    # G-build hT chunk (blocks per DMA)
GW1 = 8                     # G-build write chunk (blocks per DMA)
GCAP = 36                   # max slots per gather group
OBW = 4                     # output write batch (blocks)
POOL_EVERY = 5              # every POOL_EVERY-th gather group runs on GpSimd

_prog_cache = {}


# --------------------------------------------------------------------------- host


def _preprocess(h, src, dst):
    """Shard edges by destination, build per-core degree-sorted slot grids.

    Blocks are grouped (1-4 per gather) with a uniform slot count kg per group.
    Returns (groups, total_cols, per_core) where groups[i] = (b0, n, kg, off)
    and per_core[c] = (gidx [128, total_cols] int32, perm [NLOC] int64).
    """
    src = np.asarray(src).astype(np.int64)
    dst = np.asarray(dst).astype(np.int64)

    order = np.argsort(dst, kind="stable")
    dst_s = dst[order]
    src_s = src[order]
    bounds = np.searchsorted(dst_s, np.arange(NCORES + 1) * NLOC)

    cores = []
    blkmax = np.zeros((NCORES, NBLK), np.int64)
    for c in range(NCORES):
        a, b = bounds[c], bounds[c + 1]
        ldst = dst_s[a:b] - c * NLOC
        lsrc = src_s[a:b]
        deg = np.bincount(ldst, minlength=NLOC)
        perm = np.argsort(-deg, kind="stable")
        pdeg = np.zeros(NLOCP, np.int64)
        pdeg[:NLOC] = deg[perm]
        blkmax[c] = pdeg.reshape(NBLK, P).max(axis=1)
        cores.append((lsrc, deg, perm))

    K = blkmax.max(axis=0)
    K = np.maximum((K + 1) // 2 * 2, 2).astype(np.int64)  # even, >= 2

    groups = []
    b = 0
    off = 0
    while b < NBLK:
        kg = int(K[b])
        n = max(1, min(4, NBLK - b, GCAP // kg))
        groups.append((b, n, kg, off))
        off += n * kg
        b += n
    total_cols = int(off)

    col0 = np.zeros(NBLK, np.int64)
    for (b0, n, kg, goff) in groups:
        for j in range(n):
            col0[b0 + j] = goff + j * kg

    per_core = []
    for c in range(NCORES):
        lsrc, deg, perm = cores[c]
        start = np.concatenate([[0], np.cumsum(deg)])
        gidx = np.full((P, total_cols), NPADG, np.int32)
        for blk in range(NBLK):
            o = int(col0[blk])
            for p in range(P):
                i = blk * P + p
                if i >= NLOC:
                    continue
                node = int(perm[i])
                dn = int(deg[node])
                if dn:
                    gidx[p, o : o + dn] = lsrc[start[node] : start[node] + dn]
        bidx = np.full((P, NBLK), NPADG, np.int32)
        pidx = np.full(NLOCP, NPADG, np.int64)
        pidx[:NLOC] = c * NLOC + perm
        bidx[:, :] = pidx.reshape(NBLK, P).T
        per_core.append((gidx, bidx, perm))
    return groups, total_cols, per_core


# --------------------------------------------------------------------------- bass


def _build(groups, total_cols):
    f32 = mybir.dt.float32
    bf16 = mybir.dt.bfloat16
    i32 = mybir.dt.int32
    Sig = mybir.ActivationFunctionType.Sigmoid
    Relu = mybir.ActivationFunctionType.Relu
    mx = mybir.AluOpType.max
    ad = mybir.AluOpType.add
    ml = mybir.AluOpType.mult

    nc = bacc.Bacc(
        "TRN2", target_bir_lowering=False, debug=False, num_devices=NCORES
    )
    hT = nc.declare_dram_parameter("hT", [D, NPADG], bf16, isOutput=False)
    hloc = nc.declare_dram_parameter("hloc", [NLOCP, D], f32, isOutput=False)
    hTloc = nc.declare_dram_parameter("hTloc", [D, NLOCP], bf16, isOutput=False)
    gidx = nc.declare_dram_parameter("gidx", [P, total_cols], i32, isOutput=False)
    bidx = nc.declare_dram_parameter("bidx", [P, NBLK], i32, isOutput=False)
    wcat = nc.declare_dram_parameter("wcat", [D, 2 * D], bf16, isOutput=False)
    brhsw = nc.declare_dram_parameter("brhsw", [1, 4 * D], bf16, isOutput=False)
    u1 = nc.declare_dram_parameter("u1", [D, D], bf16, isOutput=False)
    u2 = nc.declare_dram_parameter("u2", [D, D], bf16, isOutput=False)
    ubr = nc.declare_dram_parameter("ubr", [1, D], bf16, isOutput=False)
    outp = nc.declare_dram_parameter("out", [NLOCP, D], f32, isOutput=True)

    G = nc.dram_tensor("G", [NPADG + 1, 2 * D], bf16)

    with TileContext(nc) as tc:
        with (
            tc.tile_pool(name="const", bufs=1) as cpool,
            tc.tile_pool(name="ht1", bufs=2) as htpool,
            tc.tile_pool(name="gs1", bufs=2) as gspool,
            tc.tile_pool(name="gat", bufs=4) as gatpool,
            tc.tile_pool(name="sc", bufs=2) as scpool,
            tc.tile_pool(name="ob", bufs=2) as obpool,
            tc.tile_pool(name="psA", bufs=4, space="PSUM") as psA,
            tc.tile_pool(name="psB", bufs=2, space="PSUM") as psB,
            tc.tile_pool(name="psC", bufs=2, space="PSUM") as psC,
        ):
            # ---- constants
            wcat_t = cpool.tile([D, 2 * D], bf16)
            nc.sync.dma_start(out=wcat_t[:], in_=wcat[:, :])
            brhsw_t = cpool.tile([1, 4 * D], bf16)
            nc.sync.dma_start(out=brhsw_t[:], in_=brhsw[:, :])
            u1_t = cpool.tile([D, D], bf16)
            nc.sync.dma_start(out=u1_t[:], in_=u1[:, :])
            u2_t = cpool.tile([D, D], bf16)
            nc.sync.dma_start(out=u2_t[:], in_=u2[:, :])
            ubr_t = cpool.tile([1, D], bf16)
            nc.sync.dma_start(out=ubr_t[:], in_=ubr[:, :])
            idx_t = cpool.tile([P, total_cols], i32)
            nc.sync.dma_start(out=idx_t[:], in_=gidx[:, :])
            bidx_t = cpool.tile([P, NBLK], i32)
            nc.sync.dma_start(out=bidx_t[:], in_=bidx[:, :])

            ones_bf = cpool.tile([1, P], bf16)
            nc.vector.memset(ones_bf[:], 1.0)
            ident = cpool.tile([P, P], bf16)
            make_identity(nc, ident[:])

            zrow = cpool.tile([1, 2 * D], bf16)
            nc.vector.memset(zrow[:], 0.0)
            nc.sync.dma_start(out=G[NPADG : NPADG + 1, :], in_=zrow[:])

            # replicated bias tile [A_b | 2*B_b | A_b | 2*B_b] (every partition)
            bias_sb = cpool.tile([P, 4 * D], f32)
            ps0 = psA.tile([P, 4 * D], f32, tag="g")
            nc.tensor.matmul(
                out=ps0[:], lhsT=ones_bf[:], rhs=brhsw_t[:],
                start=True, stop=True,
            )
            nc.vector.tensor_copy(out=bias_sb[:], in_=ps0[:])

            # ---- preload per-core local h (both layouts) once
            hloc_sb = cpool.tile([P, NBLK * D], f32)
            nc.sync.dma_start(
                out=hloc_sb[:].rearrange("p (b x) -> p b x", b=NBLK),
                in_=hloc[:, :].rearrange("(b p) x -> p b x", p=P),
            )
            hTloc_sb = cpool.tile([D, NLOCP], bf16)
            nc.sync.dma_start(out=hTloc_sb[:], in_=hTloc[:, :])

            bhloc_sb = cpool.tile([P, NBLK * D], bf16)
            cball = cpool.tile([P, NBLK * D], bf16)
            bpall = cpool.tile([P, NBLK * D], f32)
            ssqall = cpool.tile([P, NBLK], f32)
            nrm_t = cpool.tile([P, NBLK], f32)
            rn_t = cpool.tile([P, NBLK], f32)

            # ---- phase 1: G = [h @ A_w + A_b | h @ B_w + B_b]  (bf16)
            # Two node blocks share one PSUM bank; one wide bias matmul seeds
            # both; copies alternate Vector/Scalar per write group.
            gsgrp = 0
            for c0 in range(0, NBLK1, CH1):
                nb = min(CH1, NBLK1 - c0)
                ht_t = htpool.tile([D, CH1 * P], bf16, tag="ht")
                nc.sync.dma_start(
                    out=ht_t[:, : nb * P],
                    in_=hT[:, c0 * P : (c0 + nb) * P],
                )
                for g0 in range(0, nb, GW1):
                    gn = min(GW1, nb - g0)
                    gs = gspool.tile([P, GW1 * 2 * D], bf16, tag="gs")
                    for j0 in range(g0, g0 + gn, 2):
                        jn = min(2, g0 + gn - j0)
                        ps = psA.tile([P, 4 * D], f32, tag="g")
                        for j in range(j0, j0 + jn):
                            nc.tensor.matmul(
                                out=ps[:, (j - j0) * 2 * D : (j - j0 + 1) * 2 * D],
                                lhsT=ht_t[:, j * P : (j + 1) * P],
                                rhs=wcat_t[:],
                                start=True,
                                stop=True,
                            )
                        nc.vector.tensor_tensor(
                            out=gs[
                                :, (j0 - g0) * 2 * D : (j0 - g0 + jn) * 2 * D
                            ],
                            in0=ps[:, : jn * 2 * D],
                            in1=bias_sb[:, : jn * 2 * D],
                            op=mybir.AluOpType.add,
                        )
                    nc.sync.dma_start(
                        out=G[
                            (c0 + g0) * P : (c0 + g0 + gn) * P, :
                        ].rearrange("(j p) x -> p j x", p=P),
                        in_=gs[:, : gn * 2 * D].rearrange(
                            "p (j x) -> p j x", j=gn
                        ),
                    )

            # ---- phase 1b: Bh (no bias; 2*B_b lives in G's Bh half) for
            # local permuted nodes, straight into SBUF
            for b0 in range(0, NBLK, 4):
                bn = min(4, NBLK - b0)
                ps = psB.tile([P, 4 * D], f32, tag="s")
                for j in range(bn):
                    blk = b0 + j
                    nc.tensor.matmul(
                        out=ps[:, j * D : (j + 1) * D],
                        lhsT=hTloc_sb[:, blk * P : (blk + 1) * P],
                        rhs=wcat_t[:, D : 2 * D],
                        start=True,
                        stop=True,
                    )
                nc.scalar.copy(
                    out=bhloc_sb[:, b0 * D : (b0 + bn) * D], in_=ps[:, : bn * D]
                )

            # collapse phase-1 -> phase-2 deps into one barrier (sync-wait
            # slots on a single instruction are limited)
            tc.strict_bb_all_engine_barrier()

            # ---- phase 2+3 per gather group (software-pipelined: the
            # mult/tree/update of group g-1 runs while group g's sigmoid is on
            # the Scalar engine, so Vector never head-of-line blocks)
            def _finish(b0, n, kg, gt):
                v4 = gt[:].rearrange("p (n k x) -> p n k x", n=n, x=2 * D)
                ah = v4[:, :, :, 0:D]
                bh = v4[:, :, :, D : 2 * D]
                nc.vector.tensor_tensor(out=bh, in0=bh, in1=ah, op=ml)

                # segment max: in-place overlapped-halving tree along k
                k = kg
                while k > 2:
                    hh = (k + 1) // 2
                    nc.vector.tensor_tensor(
                        out=bh[:, :, 0:hh, :],
                        in0=bh[:, :, 0:hh, :],
                        in1=bh[:, :, k - hh : k, :],
                        op=mx,
                    )
                    k = hh
                cb_view = cball[:, b0 * D : (b0 + n) * D].rearrange(
                    "p (n o x) -> p n o x", n=n, o=1
                )
                nc.vector.tensor_tensor(
                    out=cb_view,
                    in0=bh[:, :, 0:1, :],
                    in1=bh[:, :, 1:2, :],
                    op=mx,
                )
                # clamp to 0 (covers no-in-edge nodes and all-negative maxes)
                nc.scalar.activation(
                    out=cball[:, b0 * D : (b0 + n) * D],
                    in_=cball[:, b0 * D : (b0 + n) * D],
                    func=Relu,
                )

                # ---- phase 3, batched per group: bundle = h@U1 + c@U2 + U_b
                ctps = psC.tile([P, 4 * D], bf16, tag="t")
                for j in range(n):
                    blk = b0 + j
                    nc.tensor.transpose(
                        out=ctps[:, j * D : (j + 1) * D],
                        in_=cball[:, blk * D : (blk + 1) * D],
                        identity=ident[:],
                    )
                ct = scpool.tile([P, 4 * D], bf16, tag="ct")
                nc.scalar.copy(out=ct[:, : n * D], in_=ctps[:, : n * D])

                bp = psB.tile([P, 4 * D], f32, tag="s")
                for j in range(n):
                    blk = b0 + j
                    sl = slice(j * D, (j + 1) * D)
                    nc.tensor.matmul(
                        out=bp[:, sl], lhsT=ones_bf[:], rhs=ubr_t[:],
                        start=True, stop=False,
                    )
                    nc.tensor.matmul(
                        out=bp[:, sl],
                        lhsT=hTloc_sb[:, blk * P : (blk + 1) * P],
                        rhs=u1_t[:],
                        start=False,
                        stop=False,
                    )
                    nc.tensor.matmul(
                        out=bp[:, sl], lhsT=ct[:, sl], rhs=u2_t[:],
                        start=False, stop=True,
                    )

                sq = scpool.tile([P, 4 * D], bf16, tag="sq")
                nc.scalar.activation(
                    out=sq[:, : n * D], in_=bp[:, : n * D],
                    func=mybir.ActivationFunctionType.Square,
                )
                nc.vector.tensor_reduce(
                    out=ssqall[:, b0 : b0 + n],
                    in_=sq[:, : n * D].rearrange("p (n x) -> p n x", n=n),
                    axis=mybir.AxisListType.X,
                    op=ad,
                )
                nc.scalar.activation(
                    out=bpall[:, b0 * D : (b0 + n) * D], in_=bp[:, : n * D],
                    func=Relu,
                )

            for (b0, n, kg, goff) in groups:
                gt = gatpool.tile([P, n * kg * 2 * D], bf16, tag="gt")
                nc.gpsimd.indirect_dma_start(
                    out=gt[:],
                    out_offset=None,
                    in_=G[:, :],
                    in_offset=bass.IndirectOffsetOnAxis(
                        ap=idx_t[:, goff : goff + n * kg], axis=0
                    ),
                )

                v4 = gt[:].rearrange("p (n k x) -> p n k x", n=n, x=2 * D)
                bh = v4[:, :, :, D : 2 * D]
                bhd = (
                    bhloc_sb[:, b0 * D : (b0 + n) * D]
                    .rearrange("p (n o x) -> p n o x", n=n, o=1)
                    .to_broadcast([P, n, kg, D])
                )

                # gate input and sigmoid for this group
                nc.vector.tensor_tensor(out=bh, in0=bh, in1=bhd, op=ad)
                nc.scalar.activation(out=bh, in_=bh, func=Sig)
                _finish(b0, n, kg, gt)

            # ---- tail: batched norm + scale + residual + output
            nc.scalar.sqrt(nrm_t[:], ssqall[:])
            nc.vector.tensor_scalar_max(nrm_t[:], nrm_t[:], EPS)
            nc.vector.reciprocal(rn_t[:], nrm_t[:])

            for o0 in range(0, NBLK, OBW):
                on = min(OBW, NBLK - o0)
                ob_t = obpool.tile([P, OBW * D], f32, tag="ob")
                for j in range(on):
                    blk = o0 + j
                    nc.vector.tensor_scalar_mul(
                        ob_t[:, j * D : (j + 1) * D],
                        bpall[:, blk * D : (blk + 1) * D],
                        rn_t[:, blk : blk + 1],
                    )
                nc.vector.tensor_tensor(
                    out=ob_t[:, : on * D],
                    in0=ob_t[:, : on * D],
                    in1=hloc_sb[:, o0 * D : (o0 + on) * D],
                    op=ad,
                )
                nc.sync.dma_start(
                    out=outp[o0 * P : (o0 + on) * P, :].rearrange(
                        "(j p) x -> p j x", p=P
                    ),
                    in_=ob_t[:, : on * D].rearrange("p (j x) -> p j x", j=on),
                )

    nc.compile()
    return nc


# --------------------------------------------------------------------------- run


def _run(inputs, trace=False):
    h = np.asarray(inputs["h"], np.float32)
    A_w = np.asarray(inputs["A_w"], np.float32)
    A_b = np.asarray(inputs["A_b"], np.float32)
    B_w = np.asarray(inputs["B_w"], np.float32)
    B_b = np.asarray(inputs["B_b"], np.float32)
    U_w = np.asarray(inputs["U_w"], np.float32)
    U_b = np.asarray(inputs["U_b"], np.float32)

    groups, total_cols, per_core = _preprocess(h, inputs["src"], inputs["dst"])

    key = (tuple(groups), total_cols)
    if key not in _prog_cache:
        _prog_cache.clear()
        _prog_cache[key] = _build(groups, total_cols)
    nc = _prog_cache[key]

    hT_bf = np.zeros((D, NPADG), BF16)
    hT_bf[:, :N] = h.T.astype(BF16)
    wcat = np.concatenate([A_w, B_w], axis=1).astype(BF16)
    brhs = np.concatenate([A_b, 2.0 * B_b])
    brhsw = np.concatenate([brhs, brhs])[None, :].astype(BF16)
    u1 = np.ascontiguousarray(U_w[:D]).astype(BF16)
    u2 = np.ascontiguousarray(U_w[D:]).astype(BF16)
    ubr = U_b[None, :].astype(BF16)

    in_maps = []
    for c in range(NCORES):
        gidx_c, bidx_c, perm = per_core[c]
        hl = np.zeros((NLOCP, D), np.float32)
        hl[:NLOC] = h[c * NLOC + perm]
        in_maps.append(
            {
                "hT": hT_bf,
                "hloc": hl,
                "hTloc": np.ascontiguousarray(hl.T).astype(BF16),
                "gidx": gidx_c,
                "bidx": bidx_c,
                "wcat": wcat,
                "brhsw": brhsw,
                "u1": u1,
                "u2": u2,
                "ubr": ubr,
            }
        )

    res = run_bass_kernel_spmd(nc, in_maps, list(range(NCORES)), trace=trace)

    out = np.empty((N, D), np.float32)
    for c in range(NCORES):
        _, _, perm = per_core[c]
        out[c * NLOC + perm] = res.results[c]["out"][:NLOC]
    return out, res


def kernel(**inputs) -> np.ndarray:
    out, _ = _run(inputs, trace=False)
    return out
